# revision 21
# baseline (speedup 1.0000x reference)
"""Trainium2 Bass kernel for nn_CausalMultiHypothesisTransformerLayer.

Sharding: 8 cores = 4 batches x 2 sequence-halves. Core r owns batch r//2,
s-positions [512*(r%2), 512*(r%2)+512) -> 512 tokens per core.

Layout: feature-major activations [E(partitions), tokens(free)]. All matmuls
run as out[f, t] = W[e, f].T @ act[e, t] with weights stationary, in float32r
(full PE rate at N>=256, ~1.4e-4 relative rounding).

Attention: scores_T[t, s] = k[d, t-chunk].T @ q[d, s] (feature-major both
sides, 4 heads row-packed in the PE array), softmax without max-subtraction
(score magnitudes ~3), denominator via an all-ones column appended to the
token-major V so the AV matmul emits sum(exp) as an extra output row.

Cross-core: two pairwise AllGathers (replica groups [[0,1],[2,3],[4,5],[6,7]])
exchange the intervention activations (for full-S K/V) and the hypothesis
outputs h (for the counterfactual branch). Block 0 of each AG output is
always s-half 0, so all reads are rank-independent.

LayerNorm: partition-dim reductions via ones-vector matmuls (col-packed),
rsqrt via bit-hack seed + 2 Newton iterations on the vector engine (keeps
the scalar engine on the `exp` activation-table set for the whole kernel).
"""
import os
import numpy as np

import concourse.bass as bass
import concourse.mybir as mybir
import concourse.tile as tile
from concourse import bacc
from concourse.bass_utils import run_bass_kernel_spmd

F32 = mybir.dt.float32
F32R = mybir.dt.float32r
BF16 = mybir.dt.bfloat16
I32 = mybir.dt.int32
AF = mybir.ActivationFunctionType
ALU = mybir.AluOpType

N_CORES = 8
E = 256
FF = 512
NH = 3
S = 1024
TL = 512          # tokens per core
EC = E // 128     # feature chunks (2)
RSQRT_MAGIC = 0x5F3759DF
EPS = 1e-5

DBG = bool(int(os.environ.get("KBDBG", "0")))

_cached = {}


# --------------------------------------------------------------------------
# device program
# --------------------------------------------------------------------------

def build_program():
    nc = bacc.Bacc("TRN2", target_bir_lowering=False, debug=False,
                   num_devices=N_CORES)

    def param(name, shape, dtype=F32R):
        return nc.declare_dram_parameter(name, list(shape), dtype, isOutput=False)

    prm = {}
    prm["x"] = param("x", [E, TL])
    f32_params = {"dis1_b", "dis2_b", "int1_b", "int2_b", "hq_b", "hk_b",
                  "ho_b", "cq_b", "ck_b", "co_b", "f1_b", "f2_b", "fu1_b",
                  "fu2_b"}
    bf16_params = {"ONESB"}
    for nm, shp in (
        ("dis1_w", [E, FF]), ("dis1_b", [FF, 1]),
        ("dis2_w", [FF, E]), ("dis2_b", [E, 1]),
        ("int1_w", [NH, E, 128]), ("int1_b", [NH, 128, 1]),
        ("int2_w", [NH, 128, E]), ("int2_b", [NH, E, 1]),
        ("hq_w", [NH, E, E]), ("hq_b", [NH, E, 1]),
        ("hk_w", [NH, E, E]), ("hk_b", [NH, E, 1]),
        ("hv_w", [NH, E, E]), ("hv_b", [NH, 1, E]),
        ("ho_w", [NH, E, E]), ("ho_b", [NH, E, 1]),
        ("cq_w", [NH, E, E]), ("cq_b", [NH, E, 1]),
        ("ck_w", [NH, E, E]), ("ck_b", [NH, E, 1]),
        ("cv_w", [NH, E, E]), ("cv_b", [NH, 1, E]),
        ("co_w", [NH, E, E]), ("co_b", [NH, E, 1]),
        ("f1_w", [E, FF]), ("f1_b", [FF, 1]),
        ("f2_w", [FF, E]), ("f2_b", [E, 1]),
        ("fu1_w", [NH * E, E]), ("fu1_b", [E, 1]),
        ("fu2_w", [E, NH]), ("fu2_b", [NH, 1]),
        ("lnA", [4, 1, E]), ("lnNG", [4, 1, E]), ("lnB", [4, 1, E]),
        ("E32C", [128, 128]), ("E64C", [128, 128]), ("SEL3", [3, 384]),
        ("ONES", [128, 512]), ("ONESB", [128, 1]),
    ):
        prm[nm] = param(nm, shp,
                        BF16 if nm in bf16_params else
                        F32 if nm in f32_params else F32R)

    prm["out"] = nc.declare_dram_parameter("out", [4, E, TL], F32R, isOutput=True)
    if DBG:
        for nm, shp in (("causal", [E, TL]), ("iv", [NH, E, TL]),
                        ("ivS", [NH, E, S]), ("h", [NH, E, TL]),
                        ("ref", [NH, E, S]), ("cfo", [NH, E, TL]),
                        ("refined", [NH, E, TL]), ("w3", [NH, TL])):
            prm["dbg_" + nm] = nc.declare_dram_parameter("dbg_" + nm, shp, F32R,
                                                         isOutput=True)

    with tile.TileContext(nc) as tc:
        with nc.allow_low_precision(reason="float32r tiles carry full fp32 bits"):
            _emit(nc, tc, {k: (v.ap() if hasattr(v, "ap") else v) for k, v in prm.items()})
    nc.finalize()
    return nc


def _emit(nc, tc, P):
    import contextlib
    ctx = contextlib.ExitStack()
    with ctx:
        sb = ctx.enter_context(tc.tile_pool(name="sb", bufs=1))
        tmp = ctx.enter_context(tc.tile_pool(name="tmp", bufs=2))
        ps = ctx.enter_context(tc.tile_pool(name="ps", bufs=1, space="PSUM"))
        dram = ctx.enter_context(tc.tile_pool(name="dram", bufs=1, space="DRAM"))

        V = nc.vector
        A = nc.scalar
        T = nc.tensor

        def psum(tag, bufs):
            return ps.tile([128, 512], F32, tag=tag,
                           name=f"ps_{tag}_{nc.next_id()}", bufs=bufs)

        def ttile(shape, tag, bufs, dtype=F32R):
            return tmp.tile(list(shape), dtype, tag=tag,
                            name=f"{tag}_{nc.next_id()}", bufs=bufs)

        # ---------- streamed weight loads ----------
        def wload(p_ap, In, Out, tag, bufs):
            t = ttile([128, (In // 128) * Out], tag, bufs)
            nc.sync.dma_start(out=t[:].rearrange("p (k o) -> p k o", o=Out),
                              in_=p_ap.rearrange("(k p) o -> p k o", p=128))
            return t

        def wslice(t, Out, kc, mc, m=128):
            return t[:, kc * Out + mc * 128: kc * Out + mc * 128 + m]

        # ---------- persistent small consts ----------
        def bload(p_ap, Pdim, name):
            t = sb.tile([128, max(1, Pdim // 128)], F32, tag=name, name=name)
            nc.sync.dma_start(out=t[:].rearrange("p (k o) -> p k o", o=1),
                              in_=p_ap.rearrange("(k p) o -> p k o", p=128))
            return t

        dis1b = bload(P["dis1_b"], FF, "dis1b")
        dis2b = bload(P["dis2_b"], E, "dis2b")
        int1w = [wload(P["int1_w"][n], E, 128, f"int1w{n}", 1) for n in range(NH)]
        int2w = [wload(P["int2_w"][n], 128, E, f"int2w{n}", 1) for n in range(NH)]
        int1b = [bload(P["int1_b"][n], 128, f"int1b{n}") for n in range(NH)]
        int2b = [bload(P["int2_b"][n], E, f"int2b{n}") for n in range(NH)]
        hqb = [bload(P["hq_b"][n], E, f"hqb{n}") for n in range(NH)]
        hkb = [bload(P["hk_b"][n], E, f"hkb{n}") for n in range(NH)]
        hob = [bload(P["ho_b"][n], E, f"hob{n}") for n in range(NH)]
        cqb = [bload(P["cq_b"][n], E, f"cqb{n}") for n in range(NH)]
        ckb = [bload(P["ck_b"][n], E, f"ckb{n}") for n in range(NH)]
        cob = [bload(P["co_b"][n], E, f"cob{n}") for n in range(NH)]
        f1b = bload(P["f1_b"], FF, "f1b")
        f2b = bload(P["f2_b"], E, "f2b")
        fu1b = bload(P["fu1_b"], E, "fu1b")
        hvb = sb.tile([1, NH * E], F32R, tag="hvb", name="hvb")
        nc.sync.dma_start(out=hvb[:], in_=P["hv_b"].rearrange("n o p -> o (n p)"))
        cvb = sb.tile([1, NH * E], F32R, tag="cvb", name="cvb")
        nc.sync.dma_start(out=cvb[:], in_=P["cv_b"].rearrange("n o p -> o (n p)"))
        fu2w = sb.tile([128, EC * NH], F32R, tag="fu2w", name="fu2w")
        nc.sync.dma_start(out=fu2w[:].rearrange("p (k o) -> p k o", o=NH),
                          in_=P["fu2_w"].rearrange("(k p) o -> p k o", p=128))
        fu2b = sb.tile([NH, 1], F32, tag="fu2b", name="fu2b")
        nc.sync.dma_start(out=fu2b[:], in_=P["fu2_b"])
        lnA = sb.tile([1, 4 * E], F32R, tag="lnA", name="lnA")
        nc.sync.dma_start(out=lnA[:], in_=P["lnA"].rearrange("i o e -> o (i e)"))
        lnNG = sb.tile([1, 4 * E], F32R, tag="lnNG", name="lnNG")
        nc.sync.dma_start(out=lnNG[:], in_=P["lnNG"].rearrange("i o e -> o (i e)"))
        lnB = sb.tile([1, 4 * E], F32R, tag="lnB", name="lnB")
        nc.sync.dma_start(out=lnB[:], in_=P["lnB"].rearrange("i o e -> o (i e)"))
        E32C = sb.tile([128, 128], F32R, tag="E32C", name="E32C")
        nc.sync.dma_start(out=E32C[:], in_=P["E32C"])
        E64C = sb.tile([128, 128], F32R, tag="E64C", name="E64C")
        nc.sync.dma_start(out=E64C[:], in_=P["E64C"])
        SEL3 = sb.tile([3, 384], F32R, tag="SEL3", name="SEL3")
        nc.sync.dma_start(out=SEL3[:], in_=P["SEL3"])
        ONES_t = sb.tile([128, 512], F32R, tag="ONES", name="ONES_t")
        nc.sync.dma_start(out=ONES_t[:], in_=P["ONES"])
        ONESB_t = sb.tile([128, 1], BF16, tag="ONESB", name="ONESB_t")
        nc.sync.dma_start(out=ONESB_t[:], in_=P["ONESB"])
        ones_red = ONES_t[:, 0:1]
        ones_b = ONES_t[0:1, 0:128]
        ones_row = ONES_t[0:1, 0:512]

        ag1_in = dram.tile([NH, EC, 128, TL], F32R, name="ag1_in")
        ag1_out = dram.tile([2, NH, EC, 128, TL], F32R, name="ag1_out")
        ag2_in = dram.tile([NH, EC, 128, TL], F32R, name="ag2_in")
        ag2_out = dram.tile([2, NH, EC, 128, TL], F32R, name="ag2_out")
        PAIRS = [[0, 1], [2, 3], [4, 5], [6, 7]]

        # ---------- helpers ----------
        def mlp_matmul(wtile, In, Out, rhs_tiles, bias_tile, act, out_tag, obufs,
                       extra_add=None):
            KC, MC = In // 128, Out // 128
            outs = []
            for mc in range(MC):
                pt = psum("mm", 2)
                for kc in range(KC):
                    T.matmul(pt[:], wslice(wtile, Out, kc, mc), rhs_tiles[kc][:],
                             start=(kc == 0), stop=(kc == KC - 1))
                o = ttile([128, 512], out_tag, obufs)
                b_ap = bias_tile[:, mc:mc + 1]
                if act == "relu":
                    V.tensor_scalar(o[:], pt[:], b_ap, 0.0, ALU.add, ALU.max)
                elif extra_add is not None:
                    t2 = ttile([128, 512], "scr", 2)
                    V.tensor_scalar(t2[:], pt[:], b_ap, None, ALU.add)
                    V.tensor_add(o[:], t2[:], extra_add[mc][:])
                else:
                    V.tensor_scalar(o[:], pt[:], b_ap, None, ALU.add)
                outs.append(o)
            return outs

        def st1(dtype=F32R):
            return ttile([1, 512], "st1", 7, dtype)

        def rsqrt1(var):
            """returns a [1,512] F32R tile = 1/sqrt(var + EPS); DVE-only."""
            vs = st1(F32)
            V.tensor_scalar(vs[:], var[:], EPS, None, ALU.add)
            yi = st1(I32)
            V.tensor_scalar(yi[:], vs[:].bitcast(I32), 1, None,
                            ALU.logical_shift_right)
            V.tensor_scalar(yi[:], yi[:], -1, RSQRT_MAGIC, ALU.mult, ALU.add)
            y = yi.bitcast(F32)
            for _ in range(2):
                y2 = st1(F32)
                V.tensor_mul(y2[:], y[:], y[:])
                V.tensor_mul(y2[:], y2[:], vs[:])
                V.tensor_scalar(y2[:], y2[:], -0.5, 1.5, ALU.mult, ALU.add)
                V.tensor_mul(y[:], y[:], y2[:])
            rs = st1()
            V.tensor_copy(rs[:], y[:])
            return rs

        def layer_norm(z_tiles, ln_idx, out_tag, obufs):
            redz = psum("sc", 4)
            redq = psum("sc", 4)
            for c in range(EC):
                zsq = ttile([128, 512], "lnsq", 2)
                V.tensor_mul(zsq[:], z_tiles[c][:], z_tiles[c][:])
                T.matmul(redz[0:1, :], ones_red[:], z_tiles[c][:],
                         start=(c == 0), stop=(c == EC - 1),
                         skip_group_check=True)
                T.matmul(redq[0:1, :], ones_red[:], zsq[:],
                         start=(c == 0), stop=(c == EC - 1),
                         skip_group_check=True)
            mu = st1(F32)
            V.tensor_scalar(mu[:], redz[0:1, :], 1.0 / E, None, ALU.mult)
            m2 = st1(F32)
            V.tensor_scalar(m2[:], redq[0:1, :], 1.0 / E, None, ALU.mult)
            var = st1(F32)
            V.tensor_mul(var[:], mu[:], mu[:])
            V.tensor_sub(var[:], m2[:], var[:])
            rs = rsqrt1(var)
            murs = st1()
            V.tensor_mul(murs[:], mu[:], rs[:])
            outs = []
            for c in range(EC):
                off = ln_idx * E + c * 128
                a_ps = psum("sc", 4)
                T.matmul(a_ps[:], lnA[:, off:off + 128], rs[:],
                         start=True, stop=True)
                c_ps = psum("av", 2)
                T.matmul(c_ps[:], lnNG[:, off:off + 128], murs[:],
                         start=True, stop=False)
                T.matmul(c_ps[:], lnB[:, off:off + 128], ones_row[:],
                         start=False, stop=True)
                o = ttile([128, 512], out_tag, obufs)
                V.tensor_mul(o[:], z_tiles[c][:], a_ps[:])
                V.tensor_add(o[:], o[:], c_ps[:])
                outs.append(o)
            return outs

        def dbg_dump(name, tiles_or_none, n=None):
            if not DBG:
                return
            ap = P["dbg_" + name]
            for c, t in enumerate(tiles_or_none):
                dst = ap[n, c * 128:(c + 1) * 128, :] if n is not None \
                    else ap[c * 128:(c + 1) * 128, :]
                nc.sync.dma_start(out=dst, in_=t[:])

        # ================= stage 1: x -> causal -> iv, q, AG1 =============
        x = [sb.tile([128, TL], F32R, tag=f"x{c}", name=f"x{c}") for c in range(EC)]
        for c in range(EC):
            nc.sync.dma_start(out=x[c][:], in_=P["x"][c * 128:(c + 1) * 128, :])

        dis1w = wload(P["dis1_w"], E, FF, "wmlp", 2)
        h1 = mlp_matmul(dis1w, E, FF, x, dis1b, "relu", "ff4", 4)
        dis2w = wload(P["dis2_w"], FF, E, "wmlp", 2)
        causal = mlp_matmul(dis2w, FF, E, h1, dis2b, None, "causal", 2)
        dbg_dump("causal", causal)

        iv = []
        q = []
        for n in range(NH):
            m1 = mlp_matmul(int1w[n], E, 128, causal, int1b[n], "relu", "mlp1", 2)
            ivn = mlp_matmul(int2w[n], 128, E, m1, int2b[n], None, "iv", 4,
                             extra_add=causal)
            iv.append(ivn)
            dbg_dump("iv", ivn, n)
            wq = wload(P["hq_w"][n], E, E, "wq", 2)
            q.append(mlp_matmul(wq, E, E, ivn, hqb[n], None, "q", 6))
            for c in range(EC):
                nc.sync.dma_start(out=ag1_in[n, c], in_=ivn[c][:])
        nc.gpsimd.collective_compute(
            "AllGather", ALU.bypass, replica_groups=PAIRS,
            ins=[ag1_in.opt()], outs=[ag1_out.opt()])

        # ================= stage 2: hyp KV + attention ====================
        h_sb = []
        for n in range(NH):
            ivS = [ttile([128, S], "bigS", 2) for _ in range(EC)]
            for c in range(EC):
                for half in range(2):
                    nc.sync.dma_start(out=ivS[c][:, half * TL:(half + 1) * TL],
                                      in_=ag1_out[half, n, c])
            dbg_dump("ivS", ivS, n)

            wk = wload(P["hk_w"][n], E, E, "wh", 3)
            wv = wload(P["hv_w"][n], E, E, "wh", 3)
            wo = wload(P["ho_w"][n], E, E, "wh", 3)

            k = [ttile([128, S], "ktile", 2) for _ in range(EC)]
            for mc in range(EC):
                for half in range(2):
                    pt = psum("mm", 2)
                    for kc in range(EC):
                        T.matmul(pt[:], wslice(wk, E, kc, mc),
                                 ivS[kc][:, half * TL:(half + 1) * TL],
                                 start=(kc == 0), stop=(kc == EC - 1))
                    V.tensor_scalar(k[mc][:, half * TL:(half + 1) * TL], pt[:],
                                    hkb[n][:, mc:mc + 1], None, ALU.add)

            v_tm = []
            for tcn in range(8):
                va = ttile([128, E], "vaug", 8, BF16)
                pt = psum("mm", 2)
                for kc in range(EC):
                    T.matmul(pt[:, 0:E], ivS[kc][:, tcn * 128:(tcn + 1) * 128],
                             wslice(wv, E, kc, 0, m=E),
                             start=(kc == 0), stop=False)
                T.matmul(pt[:, 0:E], ones_b[:], hvb[0:1, n * E:(n + 1) * E],
                         start=False, stop=True)
                V.tensor_copy(va[:], pt[:, 0:E])
                v_tm.append(va)

            o_n = []
            for g in range(2):
                av_a = psum("av", 2)
                av_b = psum("av", 2)
                dn_a = psum("mm", 2)
                dn_b = psum("mm", 2)
                for tcn in range(8):
                    ets = []
                    for j in range(4):
                        hh = g * 4 + j
                        sc = psum("sc", 4)
                        T.matmul(sc[:], k[hh // 4][32 * (hh % 4):32 * (hh % 4) + 32,
                                                   tcn * 128:(tcn + 1) * 128],
                                 q[n][hh // 4][32 * (hh % 4):32 * (hh % 4) + 32, :],
                                 start=True, stop=True, tile_position=(32 * j, 0))
                        et = ttile([128, 512], "et", 4, BF16)
                        A.activation(et[:], sc[:], AF.Exp,
                                     scale=float(1.0 / np.sqrt(32.0)))
                        ets.append(et)
                    for j in range(4):
                        hh = g * 4 + j
                        avt, pos = (av_a, 32 * j) if j < 3 else (av_b, 0)
                        T.matmul(avt[pos:pos + 32, :],
                                 v_tm[tcn][:, hh * 32:(hh + 1) * 32], ets[j][:],
                                 start=(tcn == 0), stop=(tcn == 7),
                                 tile_position=(0, pos), skip_group_check=True)
                    for j in range(4):
                        dnt, pos = (dn_a, 32 * j) if j < 3 else (dn_b, 0)
                        T.matmul(dnt[pos:pos + 1, :], ONESB_t[:, 0:1], ets[j][:],
                                 start=(tcn == 0), stop=(tcn == 7),
                                 tile_position=(0, pos), skip_group_check=True)
                dn_sb = ttile([128, 512], "sc2", 2)
                V.tensor_copy(dn_sb[:], ONES_t[:])
                for j in range(3):
                    V.reciprocal(dn_sb[32 * j:32 * j + 1, :], dn_a[32 * j:32 * j + 1, :])
                V.reciprocal(dn_sb[96:97, :], dn_b[0:1, :])
                av_sb = ttile([128, 512], "osb", 4)
                V.tensor_copy(av_sb[0:96, :], av_a[0:96, :])
                V.tensor_copy(av_sb[96:128, :], av_b[0:32, :])
                bc = psum("sc", 4)
                T.matmul(bc[:], E32C[:], dn_sb[:], start=True, stop=True)
                on = ttile([128, 512], "osb", 4)
                V.tensor_mul(on[:], av_sb[:], bc[:])
                o_n.append(on)
            hn = [sb.tile([128, 512], F32R, tag=f"h{n}{c}", name=f"h{n}{c}")
                  for c in range(EC)]
            for mc in range(EC):
                pt = psum("mm", 2)
                for kc in range(EC):
                    T.matmul(pt[:], wslice(wo, E, kc, mc), o_n[kc][:],
                             start=(kc == 0), stop=(kc == EC - 1))
                V.tensor_scalar(hn[mc][:], pt[:], hob[n][:, mc:mc + 1], None, ALU.add)
            h_sb.append(hn)
            dbg_dump("h", hn, n)

        # q_cf (local) + AG2
        qc = []
        for n in range(NH):
            wcq = wload(P["cq_w"][n], E, E, "wq", 2)
            qc.append(mlp_matmul(wcq, E, E, h_sb[n], cqb[n], None, "q", 6))
            for c in range(EC):
                nc.sync.dma_start(out=ag2_in[n, c], in_=h_sb[n][c][:])
        nc.gpsimd.collective_compute(
            "AllGather", ALU.bypass, replica_groups=PAIRS,
            ins=[ag2_in.opt()], outs=[ag2_out.opt()])

        hsum = [sb.tile([128, S], F32R, tag=f"hsum{c}", name=f"hsum{c}")
                for c in range(EC)]
        for n in range(NH):
            hS = [ttile([128, S], "bigS", 2) for _ in range(EC)]
            for c in range(EC):
                for half in range(2):
                    nc.sync.dma_start(out=hS[c][:, half * TL:(half + 1) * TL],
                                      in_=ag2_out[half, n, c])
            for c in range(EC):
                if n == 0:
                    V.tensor_copy(hsum[c][:], hS[c][:])
                else:
                    V.tensor_add(hsum[c][:], hsum[c][:], hS[c][:])

        # ================= stage 3: counterfactual branch =================
        refined = []
        for n in range(NH):
            ref = [ttile([128, S], "bigS", 2) for _ in range(EC)]
            for c in range(EC):
                for half in range(2):
                    nc.sync.dma_start(out=ref[c][:, half * TL:(half + 1) * TL],
                                      in_=ag2_out[half, n, c])
            for c in range(EC):
                V.tensor_sub(ref[c][:], hsum[c][:], ref[c][:])
            dbg_dump("ref", ref, n)

            wck = wload(P["ck_w"][n], E, E, "wc", 3)
            wcv = wload(P["cv_w"][n], E, E, "wc", 3)
            wco = wload(P["co_w"][n], E, E, "wc", 3)

            kc_t = [ttile([128, S], "ktile", 2) for _ in range(EC)]
            for mc in range(EC):
                for half in range(2):
                    pt = psum("mm", 2)
                    for kcc in range(EC):
                        T.matmul(pt[:], wslice(wck, E, kcc, mc),
                                 ref[kcc][:, half * TL:(half + 1) * TL],
                                 start=(kcc == 0), stop=(kcc == EC - 1))
                    V.tensor_scalar(kc_t[mc][:, half * TL:(half + 1) * TL], pt[:],
                                    ckb[n][:, mc:mc + 1], None, ALU.add)

            v_tm = []
            for tcn in range(8):
                va = ttile([128, E], "vaug", 8, BF16)
                pt = psum("mm", 2)
                for kcc in range(EC):
                    T.matmul(pt[:, 0:E], ref[kcc][:, tcn * 128:(tcn + 1) * 128],
                             wslice(wcv, E, kcc, 0, m=E),
                             start=(kcc == 0), stop=False)
                T.matmul(pt[:, 0:E], ones_b[:], cvb[0:1, n * E:(n + 1) * E],
                         start=False, stop=True)
                V.tensor_copy(va[:], pt[:, 0:E])
                v_tm.append(va)

            o_n = []
            for g in range(2):
                av_ps = psum("av", 2)
                dn_ps = psum("mm", 2)
                for tcn in range(8):
                    ets = []
                    for j in range(2):
                        hh = g * 2 + j
                        sc = psum("sc", 4)
                        T.matmul(sc[:], kc_t[hh // 2][64 * (hh % 2):64 * (hh % 2) + 64,
                                                      tcn * 128:(tcn + 1) * 128],
                                 qc[n][hh // 2][64 * (hh % 2):64 * (hh % 2) + 64, :],
                                 start=True, stop=True, tile_position=(64 * j, 0))
                        et = ttile([128, 512], "et", 4, BF16)
                        A.activation(et[:], sc[:], AF.Exp, scale=0.125)
                        ets.append(et)
                    for j in range(2):
                        hh = g * 2 + j
                        T.matmul(av_ps[64 * j:64 * j + 64, :],
                                 v_tm[tcn][:, hh * 64:(hh + 1) * 64], ets[j][:],
                                 start=(tcn == 0), stop=(tcn == 7),
                                 tile_position=(0, 64 * j), skip_group_check=True)
                    for j in range(2):
                        T.matmul(dn_ps[64 * j:64 * j + 1, :], ONESB_t[:, 0:1], ets[j][:],
                                 start=(tcn == 0), stop=(tcn == 7),
                                 tile_position=(0, 64 * j), skip_group_check=True)
                dn_sb = ttile([128, 512], "sc2", 2)
                V.tensor_copy(dn_sb[:], ONES_t[:])
                for j in range(2):
                    V.reciprocal(dn_sb[64 * j:64 * j + 1, :], dn_ps[64 * j:64 * j + 1, :])
                av_sb = ttile([128, 512], "osb", 4)
                V.tensor_copy(av_sb[:], av_ps[:])
                bc = psum("sc", 4)
                T.matmul(bc[:], E64C[:], dn_sb[:], start=True, stop=True)
                on = ttile([128, 512], "osb", 4)
                V.tensor_mul(on[:], av_sb[:], bc[:])
                o_n.append(on)
            z = []
            for mc in range(EC):
                pt = psum("mm", 2)
                for kcc in range(EC):
                    T.matmul(pt[:], wslice(wco, E, kcc, mc), o_n[kcc][:],
                             start=(kcc == 0), stop=(kcc == EC - 1))
                cfo = ttile([128, 512], "scr", 2)
                V.tensor_scalar(cfo[:], pt[:], cob[n][:, mc:mc + 1], None, ALU.add)
                if DBG:
                    nc.sync.dma_start(out=P["dbg_cfo"][n, mc * 128:(mc + 1) * 128, :],
                                      in_=cfo[:])
                zc = ttile([128, 512], "ztile", 2)
                V.tensor_add(zc[:], h_sb[n][mc][:], cfo[:])
                z.append(zc)
            rfn = layer_norm(z, 0, "lnout", 2)
            rfn_p = [sb.tile([128, 512], F32R, tag=f"rfn{n}{c}", name=f"rfn{n}{c}")
                     for c in range(EC)]
            for c in range(EC):
                V.tensor_copy(rfn_p[c][:], rfn[c][:])
                nc.sync.dma_start(out=P["out"][n, c * 128:(c + 1) * 128, :],
                                  in_=rfn_p[c][:])
            dbg_dump("refined", rfn_p, n)
            refined.append(rfn_p)

        # ================= stage 4: fusion + FFN ==========================
        fu1w_a = wload(P["fu1_w"][0:384, :], 384, E, "wmlp", 2)
        fu1w_b = wload(P["fu1_w"][384:768, :], 384, E, "wmlp", 2)
        ff1 = []
        for mc in range(EC):
            pt = psum("mm", 2)
            for kidx in range(NH * EC):
                wt, kk = (fu1w_a, kidx) if kidx < 3 else (fu1w_b, kidx - 3)
                T.matmul(pt[:], wslice(wt, E, kk, mc),
                         refined[kidx // EC][kidx % EC][:],
                         start=(kidx == 0), stop=(kidx == NH * EC - 1))
            o = ttile([128, 512], "scr", 2)
            V.tensor_scalar(o[:], pt[:], fu1b[:, mc:mc + 1], 0.0, ALU.add, ALU.max)
            ff1.append(o)
        lg = psum("mm", 2)
        for kcc in range(EC):
            T.matmul(lg[0:NH, :], fu2w[:, kcc * NH:(kcc + 1) * NH], ff1[kcc][:],
                     start=(kcc == 0), stop=(kcc == EC - 1))
        e3 = ttile([3, 512], "st1", 7)
        A.activation(e3[:], lg[0:NH, :], AF.Exp, bias=fu2b[:, 0:1])
        d3_ps = psum("av", 2)
        T.matmul(d3_ps[0:1, :], ONES_t[0:3, 0:1], e3[:], start=True, stop=True)
        r3 = st1()
        V.reciprocal(r3[:], d3_ps[0:1, :])
        bc3 = psum("av", 2)
        T.matmul(bc3[0:NH, :], ONES_t[0:1, 0:NH], r3[:], start=True, stop=True)
        w3 = ttile([3, 512], "st1", 7)
        V.tensor_mul(w3[:], e3[:], bc3[0:NH, :])
        if DBG:
            nc.sync.dma_start(out=P["dbg_w3"][:, :], in_=w3[:])

        bcw = []
        for kk in range(NH):
            bw = psum("sc", 4)
            T.matmul(bw[:], SEL3[:, 128 * kk:128 * (kk + 1)], w3[:],
                     start=True, stop=True)
            bcw.append(bw)
        mix = []
        for c in range(EC):
            acc = ttile([128, 512], "ztile", 2)
            V.tensor_mul(acc[:], refined[0][c][:], bcw[0][:])
            t2 = ttile([128, 512], "lnsq", 2)
            V.tensor_mul(t2[:], refined[1][c][:], bcw[1][:])
            V.tensor_add(acc[:], acc[:], t2[:])
            V.tensor_mul(t2[:], refined[2][c][:], bcw[2][:])
            V.tensor_add(acc[:], acc[:], t2[:])
            V.tensor_add(acc[:], acc[:], x[c][:])
            mix.append(acc)
        fused = layer_norm(mix, 1, "fused", 2)
        y = layer_norm(fused, 2, "lnout", 2)
        f1w = wload(P["f1_w"], E, FF, "wmlp", 2)
        g1 = mlp_matmul(f1w, E, FF, y, f1b, "relu", "ff4", 4)
        f2w = wload(P["f2_w"], FF, E, "wmlp", 2)
        g2 = mlp_matmul(f2w, FF, E, g1, f2b, None, "scr", 2)
        zf = []
        for c in range(EC):
            zc = ttile([128, 512], "ztile", 2)
            V.tensor_add(zc[:], fused[c][:], g2[c][:])
            zf.append(zc)
        final = layer_norm(zf, 3, "lnout", 2)
        for c in range(EC):
            nc.sync.dma_start(out=P["out"][3, c * 128:(c + 1) * 128, :],
                              in_=final[c][:])


# --------------------------------------------------------------------------
# host wrapper
# --------------------------------------------------------------------------

def _prep_shared(inp):
    f32 = lambda a: np.ascontiguousarray(np.asarray(a), dtype=np.float32)
    sh = {}
    sh["dis1_w"] = f32(inp["dis_w1"][0])
    sh["dis1_b"] = f32(inp["dis_b1"][0]).reshape(FF, 1)
    sh["dis2_w"] = f32(inp["dis_w2"][0])
    sh["dis2_b"] = f32(inp["dis_b2"][0]).reshape(E, 1)
    sh["int1_w"] = f32(inp["int_w1"])
    sh["int1_b"] = f32(inp["int_b1"]).reshape(NH, 128, 1)
    sh["int2_w"] = f32(inp["int_w2"])
    sh["int2_b"] = f32(inp["int_b2"]).reshape(NH, E, 1)
    hin_w = f32(inp["hyp_in_w"]); hin_b = f32(inp["hyp_in_b"])
    sh["hq_w"] = f32(np.transpose(hin_w[:, :E, :], (0, 2, 1)))
    sh["hk_w"] = f32(np.transpose(hin_w[:, E:2 * E, :], (0, 2, 1)))
    sh["hv_w"] = f32(np.transpose(hin_w[:, 2 * E:, :], (0, 2, 1)))
    sh["hq_b"] = hin_b[:, :E].reshape(NH, E, 1).copy()
    sh["hk_b"] = hin_b[:, E:2 * E].reshape(NH, E, 1).copy()
    sh["hv_b"] = hin_b[:, 2 * E:].reshape(NH, 1, E).copy()
    sh["ho_w"] = f32(np.transpose(f32(inp["hyp_out_w"]), (0, 2, 1)))
    sh["ho_b"] = f32(inp["hyp_out_b"]).reshape(NH, E, 1)
    cin_w = f32(inp["cf_in_w"]); cin_b = f32(inp["cf_in_b"])
    sh["cq_w"] = f32(np.transpose(cin_w[:, :E, :], (0, 2, 1)))
    sh["ck_w"] = f32(0.5 * np.transpose(cin_w[:, E:2 * E, :], (0, 2, 1)))
    sh["cv_w"] = f32(0.5 * np.transpose(cin_w[:, 2 * E:, :], (0, 2, 1)))
    sh["cq_b"] = cin_b[:, :E].reshape(NH, E, 1).copy()
    sh["ck_b"] = cin_b[:, E:2 * E].reshape(NH, E, 1).copy()
    sh["cv_b"] = cin_b[:, 2 * E:].reshape(NH, 1, E).copy()
    sh["co_w"] = f32(np.transpose(f32(inp["cf_out_w"]), (0, 2, 1)))
    sh["co_b"] = f32(inp["cf_out_b"]).reshape(NH, E, 1)
    sh["f1_w"] = f32(inp["ffn_w1"])
    sh["f1_b"] = f32(inp["ffn_b1"]).reshape(FF, 1)
    sh["f2_w"] = f32(inp["ffn_w2"])
    sh["f2_b"] = f32(inp["ffn_b2"]).reshape(E, 1)
    sh["fu1_w"] = f32(inp["fus_w1"])
    sh["fu1_b"] = f32(inp["fus_b1"]).reshape(E, 1)
    sh["fu2_w"] = f32(inp["fus_w2"])
    sh["fu2_b"] = f32(inp["fus_b2"]).reshape(NH, 1)
    g = f32(inp["norm_g"]); b = f32(inp["norm_b"])
    sh["lnA"] = g.reshape(4, 1, E).copy()
    sh["lnNG"] = (-g).reshape(4, 1, E).copy()
    sh["lnB"] = b.reshape(4, 1, E).copy()
    e32 = np.zeros((128, 128), np.float32)
    for m in range(128):
        e32[32 * (m // 32), m] = 1.0
    sh["E32C"] = e32
    e64 = np.zeros((128, 128), np.float32)
    for m in range(128):
        e64[64 * (m // 64), m] = 1.0
    sh["E64C"] = e64
    sh["ONES"] = np.ones((128, 512), np.float32)
    import ml_dtypes
    sh["ONESB"] = np.ones((128, 1), ml_dtypes.bfloat16)
    sel3 = np.zeros((3, 384), np.float32)
    for kk in range(3):
        sel3[kk, 128 * kk:128 * (kk + 1)] = 1.0
    sh["SEL3"] = sel3
    return sh


def kernel(**inputs):
    inp = {k: np.asarray(v) for k, v in inputs.items()}
    sh = _prep_shared(inp)
    x = np.ascontiguousarray(inp["x"], dtype=np.float32)   # [B, E, H, W]
    B = x.shape[0]
    x_flat = x.reshape(B, E, S)
    in_maps = []
    for r in range(N_CORES):
        b, half = r // 2, r % 2
        m = dict(sh)
        m["x"] = np.ascontiguousarray(x_flat[b][:, half * TL:(half + 1) * TL])
        in_maps.append(m)

    if "nc" not in _cached:
        _cached["nc"] = build_program()
    trace = bool(int(os.environ.get("KBTRACE", "0")))
    res = run_bass_kernel_spmd(_cached["nc"], in_maps, list(range(N_CORES)),
                               trace=trace)
    kernel.last_results = res

    full = np.zeros((4, B, E, S), np.float32)
    for r in range(N_CORES):
        b, half = r // 2, r % 2
        full[:, b, :, half * TL:(half + 1) * TL] = res.results[r]["out"]
    return tuple(np.ascontiguousarray(full[k].reshape(B, E, 32, 32))
                 for k in range(4))
